# revision 9
# baseline (speedup 1.0000x reference)
"""GQA attention (B=1, S=2048, E=4096, H=32 q-heads, KV=8 kv-heads, D=128)
sharded tensor-parallel over heads across 8 TRN2 NeuronCores.

Per core m: 4 query heads (4m..4m+3) and 1 kv head (m). Attention is fully
local per core; the host concatenates the per-core [S, 4*D] outputs.

Device kernel per core:
  phase 1: QKV projection (fp32r matmuls, contraction E=4096), bias + rope
           on DVE, producing qT/kT in [D, S] layout and v in [S, D] (via PE
           transpose) augmented with a ones column (bf16).
  phase 2: per (head, s-block of 512):
           MM1   scoresT[t,s] = kT.T @ qT   (fp32r)
           exp   ACT: expT = exp(scoresT/sqrt(D)) -> bf16
           MM2   out[s, 0:128] + denominator in col 128:
                 psum[s,129] += expT[:,t,s-tile].T @ [v_t | 1]  (bf16)
           normalize rows by 1/denominator (DVE), DMA out.
"""

import numpy as np

import concourse.bacc as bacc
import concourse.bass as bass
import concourse.mybir as mybir
import concourse.tile as tile
from concourse import bass_utils
from concourse.masks import make_identity

S = 2048
E = 4096
D = 128
HL = 4        # local query heads per core
NCORES = 8
EC = E // 128          # 32 contraction chunks
SC = S // 512          # 4 s-chunks (projection, s-blocks in attention)
TC = S // 128          # 16 t-chunks
ODIM = HL * D + 2 * D  # 768: [q0 q1 q2 q3 k v]
OB = ODIM // 128       # 6 outdim blocks
INV_SQRT_D = 1.0 / float(np.sqrt(D))

F32 = mybir.dt.float32
F32R = mybir.dt.float32r
BF16 = mybir.dt.bfloat16


def emit_preamble(nc, persist, bT, cosT, sinT):
    qkT = persist.tile([128, HL + 1, S], F32R, name="qkT")     # [d, head|k, s]
    v_sb = persist.tile([128, TC, 132], BF16, name="v_sb")     # [t%128, tchunk, d+1 pad]
    ident = persist.tile([128, 128], F32, name="ident")
    bias_sb = persist.tile([128, OB], F32, name="bias_sb")     # per-d bias, col=odim blk
    cos_sb = persist.tile([128, S], F32, name="cos_sb")        # duplicated halves
    sin_sb = persist.tile([128, S], F32, name="sin_sb")

    make_identity(nc, ident[:])
    nc.sync.dma_start(bias_sb[:], bT.ap())
    nc.sync.dma_start(cos_sb[:], cosT.ap())   # [128,S]: cos duplicated halves
    nc.sync.dma_start(sin_sb[:], sinT.ap())   # [128,S]: [-sin; +sin]
    nc.vector.memset(v_sb[:], 0.0)
    for t in range(TC):
        nc.vector.memset(v_sb[:, t, 128:129], 1.0)
    return dict(qkT=qkT, v_sb=v_sb, ident=ident, bias_sb=bias_sb,
                cos_sb=cos_sb, sin_sb=sin_sb)


def emit_body(nc, tc_, pt, xT, wqkv, out, it=0):
    """Emit one full forward pass. pt: persistent tiles from emit_preamble."""
    qkT, v_sb, ident = pt["qkT"], pt["v_sb"], pt["ident"]
    bias_sb, cos_sb, sin_sb = pt["bias_sb"], pt["cos_sb"], pt["sin_sb"]

    # ---- phase 1: QKV projection -----------------------------------------
    with (
        tc_.tile_pool(name="ph1", bufs=1) as ph1,
        tc_.tile_pool(name="xs", bufs=3) as xs,
        tc_.tile_pool(name="rot", bufs=1) as rot,
        tc_.tile_pool(name="ps1", bufs=1, space="PSUM") as ps1,
    ):
        w_sb = ph1.tile([128, EC, ODIM], F32R, name="w_sb")
        nc.sync.dma_start(w_sb[:], wqkv.ap().rearrange("(ko p) o -> p ko o", p=128))
        for j in range(SC):
            sj = slice(j * 512, (j + 1) * 512)
            psums = [
                ps1.tile([128, 512], F32, tag=f"qkv{o}", name=f"psum_{o}_{j}_{it}")
                for o in range(OB)
            ]
            for e in range(EC):
                xt = xs.tile([128, 512], F32R, tag="xt", name=f"xt_{e}_{j}_{it}")
                nc.sync.dma_start(xt[:], xT.ap()[e * 128:(e + 1) * 128, sj])
                for o in range(OB):
                    nc.tensor.matmul(
                        psums[o][:],
                        w_sb[:, e, o * 128:(o + 1) * 128],
                        xt[:],
                        start=(e == 0),
                        stop=(e == EC - 1),
                    )
            # epilogue: bias + rope for q heads and k; bias + transpose for v
            for o in range(HL + 1):  # q0..q3, k
                t12 = rot.tile([128, 512], F32, tag="t12", name=f"t12_{o}_{j}_{it}")
                nc.vector.tensor_scalar_add(t12[:], psums[o][:], bias_sb[:, o:o + 1])
                # swap partition halves via sbuf->sbuf DMA (DVE cannot cross banks)
                ts = rot.tile([128, 512], F32, tag="ts", name=f"ts_{o}_{j}_{it}")
                nc.sync.dma_start(ts[0:64], t12[64:128])
                nc.sync.dma_start(ts[64:128], t12[0:64])
                rot1 = rot.tile([128, 512], F32, tag="rot1", name=f"rot1_{o}_{j}_{it}")
                rot2 = rot.tile([128, 512], F32, tag="rot2", name=f"rot2_{o}_{j}_{it}")
                nc.vector.tensor_mul(rot1[:], t12[:], cos_sb[:, sj])
                nc.vector.tensor_mul(rot2[:], ts[:], sin_sb[:, sj])
                nc.vector.tensor_tensor(
                    qkT[:, o, sj], rot1[:], rot2[:], mybir.AluOpType.add
                )
            # v: bias then transpose [d, s] -> [s, d] in 128-blocks
            vT = rot.tile([128, 512], F32, tag="vT", name=f"vT_{j}_{it}")
            nc.vector.tensor_scalar_add(vT[:], psums[OB - 1][:], bias_sb[:, OB - 1:OB])
            for b in range(4):
                tps = ps1.tile([128, 128], F32, tag="tps", name=f"tps_{j}_{b}_{it}")
                nc.tensor.transpose(tps[:], vT[:, b * 128:(b + 1) * 128], ident[:])
                nc.vector.tensor_copy(v_sb[:, j * 4 + b, 0:128], tps[:])

    # ---- phase 2: attention ----------------------------------------------
    with (
        tc_.tile_pool(name="att", bufs=2) as att,
        tc_.tile_pool(name="ps2", bufs=3, space="PSUM") as ps2,
    ):
        for h in range(HL):
            for j in range(SC):
                sj = slice(j * 512, (j + 1) * 512)
                expT = att.tile([128, TC, 512], BF16, tag="expT",
                                name=f"expT_{h}_{j}_{it}", bufs=1)
                for t in range(TC):
                    ps_s = ps2.tile([128, 512], F32, tag="mm1",
                                    name=f"ps_s_{h}_{j}_{t}_{it}")
                    nc.tensor.matmul(
                        ps_s[:],
                        qkT[:, HL, t * 128:(t + 1) * 128],
                        qkT[:, h, sj],
                        start=True,
                        stop=True,
                    )
                    nc.scalar.activation(
                        expT[:, t, :], ps_s[:],
                        mybir.ActivationFunctionType.Exp, scale=INV_SQRT_D,
                    )
                for st in range(4):
                    ps_o = ps2.tile([128, 132], F32, tag="mm2",
                                    name=f"ps_o_{h}_{j}_{st}_{it}")
                    for t in range(TC):
                        nc.tensor.matmul(
                            ps_o[:, 0:129],
                            expT[:, t, st * 128:(st + 1) * 128],
                            v_sb[:, t, 0:129],
                            start=(t == 0),
                            stop=(t == TC - 1),
                        )
                    recip = att.tile([128, 1], F32, tag="recip",
                                     name=f"recip_{h}_{j}_{st}_{it}")
                    nc.vector.reciprocal(recip[:], ps_o[:, 128:129])
                    o_sb = att.tile([128, 128], F32, tag="o_sb",
                                    name=f"o_sb_{h}_{j}_{st}_{it}")
                    nc.vector.tensor_scalar_mul(o_sb[:], ps_o[:, 0:128], recip[:])
                    srow = (j * 4 + st) * 128
                    nc.sync.dma_start(
                        out.ap()[srow:srow + 128, h * 128:(h + 1) * 128], o_sb[:]
                    )


def build_nc(reps: int = 1, use_loop: bool = False):
    nc = bacc.Bacc("TRN2", target_bir_lowering=False, debug=False)
    xT = nc.dram_tensor("xT", (E, S), F32R, kind="ExternalInput")
    wqkv = nc.dram_tensor("wqkv", (E, ODIM), F32R, kind="ExternalInput")
    bT = nc.dram_tensor("bT", (128, OB), F32, kind="ExternalInput")
    cosT = nc.dram_tensor("cosT", (128, S), F32, kind="ExternalInput")
    sinT = nc.dram_tensor("sinT", (128, S), F32, kind="ExternalInput")
    out = nc.dram_tensor("out", (S, HL * D), F32, kind="ExternalOutput")

    with tile.TileContext(nc) as tc_:
        with tc_.tile_pool(name="persist", bufs=1) as persist:
            pt = emit_preamble(nc, persist, bT, cosT, sinT)
            if use_loop and reps > 1:
                with tc_.For_i(0, reps, 1):
                    emit_body(nc, tc_, pt, xT, wqkv, out)
            else:
                for it in range(reps):
                    emit_body(nc, tc_, pt, xT, wqkv, out, it=it)
    nc.compile()
    return nc


def make_in_maps(x, wq, bq, wk, bk, wv, bv):
    """Host-side sharding: returns per-core input dicts."""
    B, S_, E_ = x.shape
    xT = np.ascontiguousarray(x.reshape(S_, E_).T).astype(np.float32)
    inv_freq = 1.0 / (10000.0 ** (np.arange(0, D, 2, dtype=np.float32) / D))
    ang = np.arange(S_, dtype=np.float32)[:, None] * inv_freq[None, :]
    cos_t = np.cos(ang).T  # [64, S]
    sin_t = np.sin(ang).T
    cosT = np.ascontiguousarray(np.vstack([cos_t, cos_t])).astype(np.float32)
    sinT = np.ascontiguousarray(np.vstack([-sin_t, sin_t])).astype(np.float32)

    in_maps = []
    for m in range(NCORES):
        qs = slice(m * HL * D, (m + 1) * HL * D)
        ks = slice(m * D, (m + 1) * D)
        wqkv = np.ascontiguousarray(
            np.concatenate([wq[:, qs], wk[:, ks], wv[:, ks]], axis=1)
        ).astype(np.float32)
        # bias in [d, odim_block] layout
        bT = np.empty((128, OB), np.float32)
        for hl in range(HL):
            bT[:, hl] = bq[m * HL * D + hl * D:(m * HL + hl + 1) * D]
        bT[:, HL] = bk[ks]
        bT[:, HL + 1] = bv[ks]
        in_maps.append({
            "xT": xT, "wqkv": wqkv, "bT": bT, "cosT": cosT, "sinT": sinT,
        })
    return in_maps


_NC_CACHE = {}


def kernel(x, wq, bq, wk, bk, wv, bv):
    x = np.asarray(x); wq = np.asarray(wq); bq = np.asarray(bq)
    wk = np.asarray(wk); bk = np.asarray(bk)
    wv = np.asarray(wv); bv = np.asarray(bv)
    B, S_, E_ = x.shape
    if "nc" not in _NC_CACHE:
        _NC_CACHE["nc"] = build_nc()
    nc = _NC_CACHE["nc"]
    in_maps = make_in_maps(x, wq, bq, wk, bk, wv, bv)
    res = bass_utils.run_bass_kernel_spmd(nc, in_maps, core_ids=list(range(NCORES)))
    outs = [res.results[m]["out"] for m in range(NCORES)]
    full = np.concatenate(outs, axis=1).reshape(B, S_, E_).astype(np.float32)
    return full


# revision 18
# speedup vs baseline: 1.0491x; 1.0491x over previous
"""GQA attention (B=1, S=2048, E=4096, H=32 q-heads, KV=8 kv-heads, D=128)
sharded tensor-parallel over heads across 8 TRN2 NeuronCores.

Per core m: 4 query heads (4m..4m+3) and 1 kv head (m). Attention is fully
local per core; the host concatenates the per-core [S, 4*D] outputs.

Device kernel per core:
  phase 1: QKV projection (fp32r matmuls, contraction E=4096), bias + rope
           on DVE, producing qT/kT in [D, S] layout and v in [S, D] (via PE
           transpose) augmented with a ones column (bf16).
  phase 2: per (head, s-block of 512):
           MM1   scoresT[t,s] = kT.T @ qT   (fp32r)
           exp   ACT: expT = exp(scoresT/sqrt(D)) -> bf16
           MM2   out[s, 0:128] + denominator in col 128:
                 psum[s,129] += expT[:,t,s-tile].T @ [v_t | 1]  (bf16)
           normalize rows by 1/denominator (DVE), DMA out.
"""

import numpy as np

import concourse.bacc as bacc
import concourse.bass as bass
import concourse.mybir as mybir
import concourse.tile as tile
from concourse import bass_utils
from concourse.masks import make_identity

S = 2048
E = 4096
D = 128
HL = 4        # local query heads per core
NCORES = 8
EC = E // 128          # 32 contraction chunks
SC = S // 512          # 4 s-chunks (projection, s-blocks in attention)
TC = S // 128          # 16 t-chunks
ODIM = HL * D + 2 * D  # 768: [q0 q1 q2 q3 k v]
OB = ODIM // 128       # 6 outdim blocks
INV_SQRT_D = 1.0 / float(np.sqrt(D))

F32 = mybir.dt.float32
F32R = mybir.dt.float32r
BF16 = mybir.dt.bfloat16
FP16 = mybir.dt.float16
QKV_DT = F32R  # matmul dtype for projections + scores (F32R or BF16)


def emit_preamble(nc, persist, bT, cosT, sinT):
    qkT = persist.tile([128, HL + 1, S], QKV_DT, name="qkT")     # [d, head|k, s]
    v_sb = persist.tile([128, TC, 132], FP16, name="v_sb")     # [t%128, tchunk, d+1 pad]
    ident = persist.tile([128, 128], F32, name="ident")
    bias_sb = persist.tile([128, OB], F32, name="bias_sb")     # per-d bias, col=odim blk
    cos_sb = persist.tile([128, S], F32, name="cos_sb")        # duplicated halves
    sin_sb = persist.tile([128, S], F32, name="sin_sb")
    neg8 = persist.tile([128, 1], F32, name="neg8")
    nc.vector.memset(neg8[:], -8.0)

    make_identity(nc, ident[:])
    nc.sync.dma_start(bias_sb[:], bT.ap())
    nc.sync.dma_start(cos_sb[:], cosT.ap())   # [128,S]: cos duplicated halves
    nc.sync.dma_start(sin_sb[:], sinT.ap())   # [128,S]: [-sin; +sin]
    nc.vector.memset(v_sb[:], 0.0)
    for t in range(TC):
        nc.vector.memset(v_sb[:, t, 128:129], 1.0)
    return dict(qkT=qkT, v_sb=v_sb, ident=ident, bias_sb=bias_sb,
                cos_sb=cos_sb, sin_sb=sin_sb, neg8=neg8)


def emit_body(nc, tc_, pt, xT, wqkv, out, it=0):
    """Emit one full forward pass. pt: persistent tiles from emit_preamble."""
    qkT, v_sb, ident = pt["qkT"], pt["v_sb"], pt["ident"]
    bias_sb, cos_sb, sin_sb = pt["bias_sb"], pt["cos_sb"], pt["sin_sb"]
    neg8 = pt["neg8"]

    # ---- phase 1: QKV projection -----------------------------------------
    with (
        tc_.tile_pool(name="ph1", bufs=1) as ph1,
        tc_.tile_pool(name="xs", bufs=3) as xs,
        tc_.tile_pool(name="rot", bufs=1) as rot,
        tc_.tile_pool(name="ps1", bufs=1, space="PSUM") as ps1,
    ):
        w_sb = ph1.tile([128, EC, ODIM], QKV_DT, name="w_sb")
        nc.sync.dma_start(w_sb[:], wqkv.ap().rearrange("(ko p) o -> p ko o", p=128))
        for j in range(SC):
            sj = slice(j * 512, (j + 1) * 512)
            psums = [
                ps1.tile([128, 512], F32, tag=f"qkv{o}", name=f"psum_{o}_{j}_{it}")
                for o in range(OB)
            ]
            for e in range(EC):
                xt = xs.tile([128, 512], QKV_DT, tag="xt", name=f"xt_{e}_{j}_{it}")
                nc.sync.dma_start(xt[:], xT.ap()[e * 128:(e + 1) * 128, sj])
                for o in range(OB):
                    nc.tensor.matmul(
                        psums[o][:],
                        w_sb[:, e, o * 128:(o + 1) * 128],
                        xt[:],
                        start=(e == 0),
                        stop=(e == EC - 1),
                    )
            # epilogue: bias + rope for q heads and k; bias + transpose for v
            for o in range(HL + 1):  # q0..q3, k
                t12 = rot.tile([128, 512], F32, tag="t12", name=f"t12_{o}_{j}_{it}")
                # bias add on ACT (idle in phase 1) — frees the PSUM bank fast
                nc.scalar.add(t12[:], psums[o][:], bias_sb[:, o:o + 1])
                # swap partition halves via sbuf->sbuf DMA (DVE cannot cross banks)
                ts = rot.tile([128, 512], F32, tag="ts", name=f"ts_{o}_{j}_{it}")
                nc.sync.dma_start(ts[0:64], t12[64:128])
                nc.sync.dma_start(ts[64:128], t12[0:64])
                rot1 = rot.tile([128, 512], F32, tag="rot1", name=f"rot1_{o}_{j}_{it}")
                rot2 = rot.tile([128, 512], F32, tag="rot2", name=f"rot2_{o}_{j}_{it}")
                nc.vector.tensor_mul(rot1[:], t12[:], cos_sb[:, sj])
                nc.vector.tensor_mul(rot2[:], ts[:], sin_sb[:, sj])
                nc.vector.tensor_tensor(
                    qkT[:, o, sj], rot1[:], rot2[:], mybir.AluOpType.add
                )
            # v: bias then transpose [d, s] -> [s, d] in 128-blocks
            vT = rot.tile([128, 512], F32, tag="vT", name=f"vT_{j}_{it}")
            nc.scalar.add(vT[:], psums[OB - 1][:], bias_sb[:, OB - 1:OB])
            for b in range(4):
                tps = ps1.tile([128, 128], F32, tag="tps", name=f"tps_{j}_{b}_{it}")
                nc.tensor.transpose(tps[:], vT[:, b * 128:(b + 1) * 128], ident[:])
                nc.vector.tensor_copy(v_sb[:, j * 4 + b, 0:128], tps[:])

    # ---- phase 2: attention ----------------------------------------------
    with (
        tc_.tile_pool(name="att", bufs=2) as att,
        tc_.tile_pool(name="ps2", bufs=3, space="PSUM") as ps2,
    ):
        for h in range(HL):
            for j in range(SC):
                sj = slice(j * 512, (j + 1) * 512)
                expT = att.tile([128, TC, 512], FP16, tag="expT",
                                name=f"expT_{h}_{j}_{it}", bufs=2)
                for tp in range(TC // 2):
                    ps_s = ps2.tile([128, 2, 512], F32, tag="mm1",
                                    name=f"ps_s_{h}_{j}_{tp}_{it}", bufs=2)
                    for k in range(2):
                        t = tp * 2 + k
                        nc.tensor.matmul(
                            ps_s[:, k],
                            qkT[:, HL, t * 128:(t + 1) * 128],
                            qkT[:, h, sj],
                            start=True,
                            stop=True,
                        )
                    # bias=-8 keeps exp in fp16 range (softmax shift-invariant)
                    nc.scalar.activation(
                        expT[:, tp * 2:tp * 2 + 2, :], ps_s[:],
                        mybir.ActivationFunctionType.Exp, scale=INV_SQRT_D,
                        bias=neg8[:],
                    )
                for st in range(4):
                    ps_o = ps2.tile([128, 132], F32, tag="mm2",
                                    name=f"ps_o_{h}_{j}_{st}_{it}")
                    for t in range(TC):
                        nc.tensor.matmul(
                            ps_o[:, 0:129],
                            expT[:, t, st * 128:(st + 1) * 128],
                            v_sb[:, t, 0:129],
                            start=(t == 0),
                            stop=(t == TC - 1),
                        )
                    recip = att.tile([128, 1], F32, tag="recip",
                                     name=f"recip_{h}_{j}_{st}_{it}")
                    nc.vector.reciprocal(recip[:], ps_o[:, 128:129])
                    o_sb = att.tile([128, 128], F32, tag="o_sb",
                                    name=f"o_sb_{h}_{j}_{st}_{it}")
                    nc.vector.tensor_scalar_mul(o_sb[:], ps_o[:, 0:128], recip[:])
                    srow = (j * 4 + st) * 128
                    nc.sync.dma_start(
                        out.ap()[srow:srow + 128, h * 128:(h + 1) * 128], o_sb[:]
                    )


def build_nc(reps: int = 1, use_loop: bool = False):
    nc = bacc.Bacc("TRN2", target_bir_lowering=False, debug=False)
    xT = nc.dram_tensor("xT", (E, S), QKV_DT, kind="ExternalInput")
    wqkv = nc.dram_tensor("wqkv", (E, ODIM), QKV_DT, kind="ExternalInput")
    bT = nc.dram_tensor("bT", (128, OB), F32, kind="ExternalInput")
    cosT = nc.dram_tensor("cosT", (128, S), F32, kind="ExternalInput")
    sinT = nc.dram_tensor("sinT", (128, S), F32, kind="ExternalInput")
    out = nc.dram_tensor("out", (S, HL * D), F32, kind="ExternalOutput")

    with tile.TileContext(nc) as tc_:
        with tc_.tile_pool(name="persist", bufs=1) as persist:
            pt = emit_preamble(nc, persist, bT, cosT, sinT)
            if use_loop and reps > 1:
                with tc_.For_i(0, reps, 1):
                    emit_body(nc, tc_, pt, xT, wqkv, out)
            else:
                for it in range(reps):
                    emit_body(nc, tc_, pt, xT, wqkv, out, it=it)
    nc.compile()
    return nc


def make_in_maps(x, wq, bq, wk, bk, wv, bv):
    """Host-side sharding: returns per-core input dicts."""
    B, S_, E_ = x.shape
    import ml_dtypes
    host_dt = np.float32 if QKV_DT == F32R else ml_dtypes.bfloat16
    xT = np.ascontiguousarray(x.reshape(S_, E_).T).astype(host_dt)
    inv_freq = 1.0 / (10000.0 ** (np.arange(0, D, 2, dtype=np.float32) / D))
    ang = np.arange(S_, dtype=np.float32)[:, None] * inv_freq[None, :]
    cos_t = np.cos(ang).T  # [64, S]
    sin_t = np.sin(ang).T
    cosT = np.ascontiguousarray(np.vstack([cos_t, cos_t])).astype(np.float32)
    sinT = np.ascontiguousarray(np.vstack([-sin_t, sin_t])).astype(np.float32)

    in_maps = []
    for m in range(NCORES):
        qs = slice(m * HL * D, (m + 1) * HL * D)
        ks = slice(m * D, (m + 1) * D)
        wqkv = np.ascontiguousarray(
            np.concatenate([wq[:, qs], wk[:, ks], wv[:, ks]], axis=1)
        ).astype(host_dt)
        # bias in [d, odim_block] layout
        bT = np.empty((128, OB), np.float32)
        for hl in range(HL):
            bT[:, hl] = bq[m * HL * D + hl * D:(m * HL + hl + 1) * D]
        bT[:, HL] = bk[ks]
        bT[:, HL + 1] = bv[ks]
        in_maps.append({
            "xT": xT, "wqkv": wqkv, "bT": bT, "cosT": cosT, "sinT": sinT,
        })
    return in_maps


_NC_CACHE = {}


def kernel(x, wq, bq, wk, bk, wv, bv):
    x = np.asarray(x); wq = np.asarray(wq); bq = np.asarray(bq)
    wk = np.asarray(wk); bk = np.asarray(bk)
    wv = np.asarray(wv); bv = np.asarray(bv)
    B, S_, E_ = x.shape
    if "nc" not in _NC_CACHE:
        _NC_CACHE["nc"] = build_nc()
    nc = _NC_CACHE["nc"]
    in_maps = make_in_maps(x, wq, bq, wk, bk, wv, bv)
    res = bass_utils.run_bass_kernel_spmd(nc, in_maps, core_ids=list(range(NCORES)))
    outs = [res.results[m]["out"] for m in range(NCORES)]
    full = np.concatenate(outs, axis=1).reshape(B, S_, E_).astype(np.float32)
    return full


# revision 21
# speedup vs baseline: 15401.9748x; 14681.5496x over previous
"""GQA attention (B=1, S=2048, E=4096, H=32 q-heads, KV=8 kv-heads, D=128)
sharded tensor-parallel over heads across 8 TRN2 NeuronCores.

Per core m: 4 query heads (4m..4m+3) and 1 kv head (m). Attention is fully
local per core; the host concatenates the per-core [S, 4*D] outputs.

Device kernel per core:
  phase 1: QKV projection (fp32r matmuls, contraction E=4096), bias + rope
           on DVE, producing qT/kT in [D, S] layout and v in [S, D] (via PE
           transpose) augmented with a ones column (fp16).
  phase 2: per (head, s-block of 512):
           MM1   scoresT[t,s] = kT.T @ qT   (fp32r)
           exp   ACT: expT = exp(scoresT/sqrt(D) - 8) -> fp16
           MM2   out[s, 0:128] + denominator in col 128:
                 psum[s,129] += expT[:,t,s-tile].T @ [v_t | 1]  (fp16)
           normalize rows by 1/denominator (DVE), DMA out.
"""

import numpy as np

import concourse.bacc as bacc
import concourse.bass as bass
import concourse.mybir as mybir
import concourse.tile as tile
from concourse import bass_utils
from concourse.masks import make_identity

S = 2048
E = 4096
D = 128
HL = 4        # local query heads per core
NCORES = 8
EC = E // 128          # 32 contraction chunks
SC = S // 512          # 4 s-chunks (projection, s-blocks in attention)
TC = S // 128          # 16 t-chunks
ODIM = HL * D + 2 * D  # 768: [q0 q1 q2 q3 k v]
OB = ODIM // 128       # 6 outdim blocks
INV_SQRT_D = 1.0 / float(np.sqrt(D))

F32 = mybir.dt.float32
F32R = mybir.dt.float32r
BF16 = mybir.dt.bfloat16
FP16 = mybir.dt.float16
QKV_DT = F32R  # matmul dtype for projections + scores (F32R or BF16)


def emit_preamble(nc, persist, bT, cosT, sinT):
    qkT = persist.tile([128, HL + 1, S], QKV_DT, name="qkT")     # [d, head|k, s]
    v_sb = persist.tile([128, TC, 132], FP16, name="v_sb")     # [t%128, tchunk, d+1 pad]
    ident = persist.tile([128, 128], F32, name="ident")
    bias_sb = persist.tile([128, OB], F32, name="bias_sb")     # per-d bias, col=odim blk
    cos_sb = persist.tile([128, S], F32, name="cos_sb")        # duplicated halves
    sin_sb = persist.tile([128, S], F32, name="sin_sb")
    neg8 = persist.tile([128, 1], F32, name="neg8")
    nc.vector.memset(neg8[:], -8.0)

    make_identity(nc, ident[:])
    nc.sync.dma_start(bias_sb[:], bT.ap())
    nc.sync.dma_start(cos_sb[:], cosT.ap())   # [128,S]: cos duplicated halves
    nc.sync.dma_start(sin_sb[:], sinT.ap())   # [128,S]: [-sin; +sin]
    nc.vector.memset(v_sb[:], 0.0)
    for t in range(TC):
        nc.vector.memset(v_sb[:, t, 128:129], 1.0)
    return dict(qkT=qkT, v_sb=v_sb, ident=ident, bias_sb=bias_sb,
                cos_sb=cos_sb, sin_sb=sin_sb, neg8=neg8)


def emit_body(nc, tc_, pt, xT, wqkv, out, it=0):
    """Emit one full forward pass. pt: persistent tiles from emit_preamble."""
    qkT, v_sb, ident = pt["qkT"], pt["v_sb"], pt["ident"]
    bias_sb, cos_sb, sin_sb = pt["bias_sb"], pt["cos_sb"], pt["sin_sb"]
    neg8 = pt["neg8"]

    # ---- phase 1: QKV projection -----------------------------------------
    with (
        tc_.tile_pool(name="ph1", bufs=1) as ph1,
        tc_.tile_pool(name="xs", bufs=4) as xs,
        tc_.tile_pool(name="rot", bufs=2) as rot,
        tc_.tile_pool(name="ps1", bufs=1, space="PSUM") as ps1,
    ):
        w_sb = ph1.tile([128, EC, ODIM], QKV_DT, name="w_sb")
        nc.sync.dma_start(w_sb[:], wqkv.ap().rearrange("(ko p) o -> p ko o", p=128))
        for j in range(SC):
            sj = slice(j * 512, (j + 1) * 512)
            psums = [
                ps1.tile([128, 512], F32, tag=f"qkv{o}", name=f"psum_{o}_{j}_{it}")
                for o in range(OB)
            ]
            for e in range(EC):
                xt = xs.tile([128, 512], QKV_DT, tag="xt", name=f"xt_{e}_{j}_{it}")
                nc.sync.dma_start(xt[:], xT.ap()[e * 128:(e + 1) * 128, sj])
                for o in range(OB):
                    nc.tensor.matmul(
                        psums[o][:],
                        w_sb[:, e, o * 128:(o + 1) * 128],
                        xt[:],
                        start=(e == 0),
                        stop=(e == EC - 1),
                    )
            # epilogue: bias + rope for q heads and k; bias + transpose for v
            for o in range(HL + 1):  # q0..q3, k
                t12 = rot.tile([128, 512], F32, tag="t12", name=f"t12_{o}_{j}_{it}")
                # bias add on ACT (idle in phase 1) — frees the PSUM bank fast
                nc.scalar.add(t12[:], psums[o][:], bias_sb[:, o:o + 1])
                # swap partition halves via sbuf->sbuf DMA (DVE cannot cross banks)
                ts = rot.tile([128, 512], F32, tag="ts", name=f"ts_{o}_{j}_{it}")
                nc.sync.dma_start(ts[0:64], t12[64:128])
                nc.sync.dma_start(ts[64:128], t12[0:64])
                rot1 = rot.tile([128, 512], F32, tag="rot1", name=f"rot1_{o}_{j}_{it}")
                rot2 = rot.tile([128, 512], F32, tag="rot2", name=f"rot2_{o}_{j}_{it}")
                nc.vector.tensor_mul(rot1[:], t12[:], cos_sb[:, sj])
                nc.vector.tensor_mul(rot2[:], ts[:], sin_sb[:, sj])
                nc.vector.tensor_tensor(
                    qkT[:, o, sj], rot1[:], rot2[:], mybir.AluOpType.add
                )
            # v: bias then transpose [d, s] -> [s, d] in 128-blocks
            vT = rot.tile([128, 512], F32, tag="vT", name=f"vT_{j}_{it}")
            nc.scalar.add(vT[:], psums[OB - 1][:], bias_sb[:, OB - 1:OB])
            for b in range(4):
                tps = ps1.tile([128, 128], F32, tag="tps", name=f"tps_{j}_{b}_{it}")
                nc.tensor.transpose(tps[:], vT[:, b * 128:(b + 1) * 128], ident[:])
                nc.vector.tensor_copy(v_sb[:, j * 4 + b, 0:128], tps[:])

    # ---- phase 2: attention ----------------------------------------------
    with (
        tc_.tile_pool(name="att", bufs=2) as att,
        tc_.tile_pool(name="ps2", bufs=3, space="PSUM") as ps2,
    ):
        for h in range(HL):
            for j in range(SC):
                sj = slice(j * 512, (j + 1) * 512)
                expT = att.tile([128, TC, 512], FP16, tag="expT",
                                name=f"expT_{h}_{j}_{it}", bufs=2)
                for tp in range(TC // 2):
                    ps_s = ps2.tile([128, 2, 512], F32, tag="mm1",
                                    name=f"ps_s_{h}_{j}_{tp}_{it}", bufs=2)
                    for k in range(2):
                        t = tp * 2 + k
                        nc.tensor.matmul(
                            ps_s[:, k],
                            qkT[:, HL, t * 128:(t + 1) * 128],
                            qkT[:, h, sj],
                            start=True,
                            stop=True,
                        )
                    # bias=-8 keeps exp in fp16 range (softmax shift-invariant)
                    nc.scalar.activation(
                        expT[:, tp * 2:tp * 2 + 2, :], ps_s[:],
                        mybir.ActivationFunctionType.Exp, scale=INV_SQRT_D,
                        bias=neg8[:],
                    )
                for st in range(4):
                    ps_o = ps2.tile([128, 132], F32, tag="mm2",
                                    name=f"ps_o_{h}_{j}_{st}_{it}")
                    for t in range(TC):
                        nc.tensor.matmul(
                            ps_o[:, 0:129],
                            expT[:, t, st * 128:(st + 1) * 128],
                            v_sb[:, t, 0:129],
                            start=(t == 0),
                            stop=(t == TC - 1),
                        )
                    recip = att.tile([128, 1], F32, tag="recip",
                                     name=f"recip_{h}_{j}_{st}_{it}")
                    nc.vector.reciprocal(recip[:], ps_o[:, 128:129])
                    o_sb = att.tile([128, 128], F32, tag="o_sb",
                                    name=f"o_sb_{h}_{j}_{st}_{it}")
                    nc.vector.tensor_scalar_mul(o_sb[:], ps_o[:, 0:128], recip[:])
                    srow = (j * 4 + st) * 128
                    nc.sync.dma_start(
                        out.ap()[srow:srow + 128, h * 128:(h + 1) * 128], o_sb[:]
                    )


def build_nc(reps: int = 1, use_loop: bool = False):
    nc = bacc.Bacc("TRN2", target_bir_lowering=False, debug=False)
    xT = nc.dram_tensor("xT", (E, S), QKV_DT, kind="ExternalInput")
    wqkv = nc.dram_tensor("wqkv", (E, ODIM), QKV_DT, kind="ExternalInput")
    bT = nc.dram_tensor("bT", (128, OB), F32, kind="ExternalInput")
    cosT = nc.dram_tensor("cosT", (128, S), F32, kind="ExternalInput")
    sinT = nc.dram_tensor("sinT", (128, S), F32, kind="ExternalInput")
    out = nc.dram_tensor("out", (S, HL * D), F32, kind="ExternalOutput")

    with tile.TileContext(nc) as tc_:
        with tc_.tile_pool(name="persist", bufs=1) as persist:
            pt = emit_preamble(nc, persist, bT, cosT, sinT)
            if use_loop and reps > 1:
                with tc_.For_i(0, reps, 1):
                    emit_body(nc, tc_, pt, xT, wqkv, out)
            else:
                for it in range(reps):
                    emit_body(nc, tc_, pt, xT, wqkv, out, it=it)
    nc.compile()
    return nc


def make_in_maps(x, wq, bq, wk, bk, wv, bv):
    """Host-side sharding: returns per-core input dicts."""
    B, S_, E_ = x.shape
    import ml_dtypes
    host_dt = np.float32 if QKV_DT == F32R else ml_dtypes.bfloat16
    xT = np.ascontiguousarray(x.reshape(S_, E_).T).astype(host_dt)
    inv_freq = 1.0 / (10000.0 ** (np.arange(0, D, 2, dtype=np.float32) / D))
    ang = np.arange(S_, dtype=np.float32)[:, None] * inv_freq[None, :]
    cos_t = np.cos(ang).T  # [64, S]
    sin_t = np.sin(ang).T
    cosT = np.ascontiguousarray(np.vstack([cos_t, cos_t])).astype(np.float32)
    sinT = np.ascontiguousarray(np.vstack([-sin_t, sin_t])).astype(np.float32)

    in_maps = []
    for m in range(NCORES):
        qs = slice(m * HL * D, (m + 1) * HL * D)
        ks = slice(m * D, (m + 1) * D)
        wqkv = np.ascontiguousarray(
            np.concatenate([wq[:, qs], wk[:, ks], wv[:, ks]], axis=1)
        ).astype(host_dt)
        # bias in [d, odim_block] layout
        bT = np.empty((128, OB), np.float32)
        for hl in range(HL):
            bT[:, hl] = bq[m * HL * D + hl * D:(m * HL + hl + 1) * D]
        bT[:, HL] = bk[ks]
        bT[:, HL + 1] = bv[ks]
        in_maps.append({
            "xT": xT, "wqkv": wqkv, "bT": bT, "cosT": cosT, "sinT": sinT,
        })
    return in_maps


_NC_CACHE = {}


def kernel(x, wq, bq, wk, bk, wv, bv):
    x = np.asarray(x); wq = np.asarray(wq); bq = np.asarray(bq)
    wk = np.asarray(wk); bk = np.asarray(bk)
    wv = np.asarray(wv); bv = np.asarray(bv)
    B, S_, E_ = x.shape
    if "nc" not in _NC_CACHE:
        _NC_CACHE["nc"] = build_nc()
    nc = _NC_CACHE["nc"]
    in_maps = make_in_maps(x, wq, bq, wk, bk, wv, bv)
    res = bass_utils.run_bass_kernel_spmd(nc, in_maps, core_ids=list(range(NCORES)))
    outs = [res.results[m]["out"] for m in range(NCORES)]
    full = np.concatenate(outs, axis=1).reshape(B, S_, E_).astype(np.float32)
    return full


# revision 22
# speedup vs baseline: 17018.2839x; 1.1049x over previous
"""GQA attention (B=1, S=2048, E=4096, H=32 q-heads, KV=8 kv-heads, D=128)
sharded tensor-parallel over heads across 8 TRN2 NeuronCores.

Per core m: 4 query heads (4m..4m+3) and 1 kv head (m). Attention is fully
local per core; the host concatenates the per-core [S, 4*D] outputs.

Device kernel per core:
  phase 1: QKV projection (fp32r matmuls, contraction E=4096), bias + rope
           on DVE, producing qT/kT in [D, S] layout and v in [S, D] (via PE
           transpose) augmented with a ones column (bf16).
  phase 2: per (head, s-block of 512):
           MM1   scoresT[t,s] = kT.T @ qT   (fp32r)
           exp   ACT: expT = exp(scoresT/sqrt(D)) -> bf16
           MM2   out[s, 0:128] + denominator in col 128:
                 psum[s,129] += expT[:,t,s-tile].T @ [v_t | 1]  (bf16)
           normalize rows by 1/denominator (DVE), DMA out.
"""

import numpy as np

import concourse.bacc as bacc
import concourse.bass as bass
import concourse.mybir as mybir
import concourse.tile as tile
from concourse import bass_utils
from concourse.masks import make_identity

S = 2048
E = 4096
D = 128
HL = 4        # local query heads per core
NCORES = 8
EC = E // 128          # 32 contraction chunks
SC = S // 512          # 4 s-chunks (projection, s-blocks in attention)
TC = S // 128          # 16 t-chunks
ODIM = HL * D + 2 * D  # 768: [q0 q1 q2 q3 k v]
OB = ODIM // 128       # 6 outdim blocks
INV_SQRT_D = 1.0 / float(np.sqrt(D))

F32 = mybir.dt.float32
F32R = mybir.dt.float32r
BF16 = mybir.dt.bfloat16
FP16 = mybir.dt.float16
QKV_DT = F32R  # matmul dtype for projections + scores (F32R or BF16)


def emit_preamble(nc, persist, bT, cosT, sinT):
    qkT = persist.tile([128, HL + 1, S], QKV_DT, name="qkT")     # [d, head|k, s]
    v_sb = persist.tile([128, TC, 132], FP16, name="v_sb")     # [t%128, tchunk, d+1 pad]
    ident = persist.tile([128, 128], F32, name="ident")
    bias_sb = persist.tile([128, OB], F32, name="bias_sb")     # per-d bias, col=odim blk
    cos_sb = persist.tile([128, S], F32, name="cos_sb")        # duplicated halves
    sin_sb = persist.tile([128, S], F32, name="sin_sb")
    neg8 = persist.tile([128, 1], F32, name="neg8")
    nc.vector.memset(neg8[:], -8.0)

    make_identity(nc, ident[:])
    nc.sync.dma_start(bias_sb[:], bT.ap())
    nc.sync.dma_start(cos_sb[:], cosT.ap())   # [128,S]: cos duplicated halves
    nc.sync.dma_start(sin_sb[:], sinT.ap())   # [128,S]: [-sin; +sin]
    nc.vector.memset(v_sb[:], 0.0)
    for t in range(TC):
        nc.vector.memset(v_sb[:, t, 128:129], 1.0)
    return dict(qkT=qkT, v_sb=v_sb, ident=ident, bias_sb=bias_sb,
                cos_sb=cos_sb, sin_sb=sin_sb, neg8=neg8)


def emit_body(nc, tc_, pt, xT, wqkv, out, it=0):
    """Emit one full forward pass. pt: persistent tiles from emit_preamble."""
    qkT, v_sb, ident = pt["qkT"], pt["v_sb"], pt["ident"]
    bias_sb, cos_sb, sin_sb = pt["bias_sb"], pt["cos_sb"], pt["sin_sb"]
    neg8 = pt["neg8"]

    # ---- phase 1: QKV projection -----------------------------------------
    with (
        tc_.tile_pool(name="ph1", bufs=1) as ph1,
        tc_.tile_pool(name="xs", bufs=4) as xs,
        tc_.tile_pool(name="rot", bufs=2) as rot,
        tc_.tile_pool(name="ps1", bufs=1, space="PSUM") as ps1,
    ):
        w_sb = ph1.tile([128, EC, ODIM], QKV_DT, name="w_sb")
        nc.sync.dma_start(w_sb[:], wqkv.ap().rearrange("(ko p) o -> p ko o", p=128))
        for j in range(SC):
            sj = slice(j * 512, (j + 1) * 512)
            psums = [
                ps1.tile([128, 512], F32, tag=f"qkv{o}", name=f"psum_{o}_{j}_{it}")
                for o in range(OB)
            ]
            for e in range(EC):
                xt = xs.tile([128, 512], QKV_DT, tag="xt", name=f"xt_{e}_{j}_{it}")
                nc.sync.dma_start(xt[:], xT.ap()[e * 128:(e + 1) * 128, sj])
                for o in range(OB):
                    nc.tensor.matmul(
                        psums[o][:],
                        w_sb[:, e, o * 128:(o + 1) * 128],
                        xt[:],
                        start=(e == 0),
                        stop=(e == EC - 1),
                    )
            # epilogue: bias + rope for q heads and k; bias + transpose for v
            for o in range(HL + 1):  # q0..q3, k
                t12 = rot.tile([128, 512], F32, tag="t12", name=f"t12_{o}_{j}_{it}")
                # bias add on ACT (idle in phase 1) — frees the PSUM bank fast
                nc.scalar.add(t12[:], psums[o][:], bias_sb[:, o:o + 1])
                # swap partition halves via sbuf->sbuf DMA (DVE cannot cross banks)
                ts = rot.tile([128, 512], F32, tag="ts", name=f"ts_{o}_{j}_{it}")
                nc.sync.dma_start(ts[0:64], t12[64:128])
                nc.sync.dma_start(ts[64:128], t12[0:64])
                rot1 = rot.tile([128, 512], F32, tag="rot1", name=f"rot1_{o}_{j}_{it}")
                rot2 = rot.tile([128, 512], F32, tag="rot2", name=f"rot2_{o}_{j}_{it}")
                nc.vector.tensor_mul(rot1[:], t12[:], cos_sb[:, sj])
                nc.vector.tensor_mul(rot2[:], ts[:], sin_sb[:, sj])
                nc.vector.tensor_tensor(
                    qkT[:, o, sj], rot1[:], rot2[:], mybir.AluOpType.add
                )
            # v: bias then transpose [d, s] -> [s, d] in 128-blocks
            vT = rot.tile([128, 512], F32, tag="vT", name=f"vT_{j}_{it}")
            nc.scalar.add(vT[:], psums[OB - 1][:], bias_sb[:, OB - 1:OB])
            for b in range(4):
                tps = ps1.tile([128, 128], F32, tag="tps", name=f"tps_{j}_{b}_{it}")
                nc.tensor.transpose(tps[:], vT[:, b * 128:(b + 1) * 128], ident[:])
                nc.vector.tensor_copy(v_sb[:, j * 4 + b, 0:128], tps[:])

    # ---- phase 2: attention ----------------------------------------------
    with (
        tc_.tile_pool(name="att", bufs=2) as att,
        tc_.tile_pool(name="ps2", bufs=3, space="PSUM") as ps2,
    ):
        for h in range(HL):
            for j in range(SC):
                sj = slice(j * 512, (j + 1) * 512)
                expT = att.tile([128, TC, 512], FP16, tag="expT",
                                name=f"expT_{h}_{j}_{it}", bufs=2)
                for tp in range(TC // 2):
                    ps_s = ps2.tile([128, 2, 512], F32, tag="mm1",
                                    name=f"ps_s_{h}_{j}_{tp}_{it}", bufs=3)
                    for k in range(2):
                        t = tp * 2 + k
                        nc.tensor.matmul(
                            ps_s[:, k],
                            qkT[:, HL, t * 128:(t + 1) * 128],
                            qkT[:, h, sj],
                            start=True,
                            stop=True,
                        )
                    # bias=-8 keeps exp in fp16 range (softmax shift-invariant)
                    nc.scalar.activation(
                        expT[:, tp * 2:tp * 2 + 2, :], ps_s[:],
                        mybir.ActivationFunctionType.Exp, scale=INV_SQRT_D,
                        bias=neg8[:],
                    )
                for st in range(4):
                    ps_o = ps2.tile([128, 132], F32, tag="mm2",
                                    name=f"ps_o_{h}_{j}_{st}_{it}", bufs=2)
                    for t in range(TC):
                        nc.tensor.matmul(
                            ps_o[:, 0:129],
                            expT[:, t, st * 128:(st + 1) * 128],
                            v_sb[:, t, 0:129],
                            start=(t == 0),
                            stop=(t == TC - 1),
                        )
                    recip = att.tile([128, 1], F32, tag="recip",
                                     name=f"recip_{h}_{j}_{st}_{it}")
                    nc.vector.reciprocal(recip[:], ps_o[:, 128:129])
                    o_sb = att.tile([128, 128], F32, tag="o_sb",
                                    name=f"o_sb_{h}_{j}_{st}_{it}")
                    nc.vector.tensor_scalar_mul(o_sb[:], ps_o[:, 0:128], recip[:])
                    srow = (j * 4 + st) * 128
                    nc.sync.dma_start(
                        out.ap()[srow:srow + 128, h * 128:(h + 1) * 128], o_sb[:]
                    )


def build_nc(reps: int = 1, use_loop: bool = False):
    nc = bacc.Bacc("TRN2", target_bir_lowering=False, debug=False)
    xT = nc.dram_tensor("xT", (E, S), QKV_DT, kind="ExternalInput")
    wqkv = nc.dram_tensor("wqkv", (E, ODIM), QKV_DT, kind="ExternalInput")
    bT = nc.dram_tensor("bT", (128, OB), F32, kind="ExternalInput")
    cosT = nc.dram_tensor("cosT", (128, S), F32, kind="ExternalInput")
    sinT = nc.dram_tensor("sinT", (128, S), F32, kind="ExternalInput")
    out = nc.dram_tensor("out", (S, HL * D), F32, kind="ExternalOutput")

    with tile.TileContext(nc) as tc_:
        with tc_.tile_pool(name="persist", bufs=1) as persist:
            pt = emit_preamble(nc, persist, bT, cosT, sinT)
            if use_loop and reps > 1:
                with tc_.For_i(0, reps, 1):
                    emit_body(nc, tc_, pt, xT, wqkv, out)
            else:
                for it in range(reps):
                    emit_body(nc, tc_, pt, xT, wqkv, out, it=it)
    nc.compile()
    return nc


def make_in_maps(x, wq, bq, wk, bk, wv, bv):
    """Host-side sharding: returns per-core input dicts."""
    B, S_, E_ = x.shape
    import ml_dtypes
    host_dt = np.float32 if QKV_DT == F32R else ml_dtypes.bfloat16
    xT = np.ascontiguousarray(x.reshape(S_, E_).T).astype(host_dt)
    inv_freq = 1.0 / (10000.0 ** (np.arange(0, D, 2, dtype=np.float32) / D))
    ang = np.arange(S_, dtype=np.float32)[:, None] * inv_freq[None, :]
    cos_t = np.cos(ang).T  # [64, S]
    sin_t = np.sin(ang).T
    cosT = np.ascontiguousarray(np.vstack([cos_t, cos_t])).astype(np.float32)
    sinT = np.ascontiguousarray(np.vstack([-sin_t, sin_t])).astype(np.float32)

    in_maps = []
    for m in range(NCORES):
        qs = slice(m * HL * D, (m + 1) * HL * D)
        ks = slice(m * D, (m + 1) * D)
        wqkv = np.ascontiguousarray(
            np.concatenate([wq[:, qs], wk[:, ks], wv[:, ks]], axis=1)
        ).astype(host_dt)
        # bias in [d, odim_block] layout
        bT = np.empty((128, OB), np.float32)
        for hl in range(HL):
            bT[:, hl] = bq[m * HL * D + hl * D:(m * HL + hl + 1) * D]
        bT[:, HL] = bk[ks]
        bT[:, HL + 1] = bv[ks]
        in_maps.append({
            "xT": xT, "wqkv": wqkv, "bT": bT, "cosT": cosT, "sinT": sinT,
        })
    return in_maps


_NC_CACHE = {}


def kernel(x, wq, bq, wk, bk, wv, bv):
    x = np.asarray(x); wq = np.asarray(wq); bq = np.asarray(bq)
    wk = np.asarray(wk); bk = np.asarray(bk)
    wv = np.asarray(wv); bv = np.asarray(bv)
    B, S_, E_ = x.shape
    if "nc" not in _NC_CACHE:
        _NC_CACHE["nc"] = build_nc()
    nc = _NC_CACHE["nc"]
    in_maps = make_in_maps(x, wq, bq, wk, bk, wv, bv)
    res = bass_utils.run_bass_kernel_spmd(nc, in_maps, core_ids=list(range(NCORES)))
    outs = [res.results[m]["out"] for m in range(NCORES)]
    full = np.concatenate(outs, axis=1).reshape(B, S_, E_).astype(np.float32)
    return full
